# revision 1
# baseline (speedup 1.0000x reference)
"""Trainium2 Bass kernel for BSplineBasis (degree-3, 64 uniform-ish knots).

Math: the reference evaluates, for each normalized point xn and each of 60
basis elements i, a piecewise cubic (de Boor with clamped interval index).
With simple inner knots this is exactly representable in truncated-power form:

    out[n, i] = sum_q A[q,i] * xn^q  +  sum_m J[m,i] * relu(xn - kappa_m)^3

with 56 inner-knot features kappa_m and a banded (3 taps/column) jump table J.
On device (per 490-point tile, two tiles paired via block-diagonal weights so
fp32r matmuls keep start_partition 0 and DVE/ACT passes use 120 lanes):
  MM1 (K=6, fp32r): unclamped cube polynomials + y-power pass-throughs from
      rows [y, y^2, y^3] (y = xn - 0.5, centered to tame monomial
      cancellation at tf32 precision); constant terms live in the clamp bias
  clamp (DVE): stack = max(G + bias, minclamp) with per-partition minclamp
      (-inf on the power rows so negative y passes through)
  MM2 (K=120, fp32r): out.T = blockdiag(W2, W2).T @ stack -> PSUM
  evict (ACT): PSUM -> SBUF staging, then HWDGE DMA to DRAM out_t [60, shard]
Each core redundantly computes the global min/max from the full x (no
collective needed) and processes a 62,500-point shard; the host transposes
out_t back and patches the (rare) xn == 1.0 rows where the reference jumps
to its degenerate right-end pieces.

Tables are derived from the knots at runtime on the host in float64, by
fitting the reference's own de Boor piece recursion (exact for cubics).
"""
import os
import sys

import numpy as np

if "/opt/trn_rl_repo" not in sys.path:
    sys.path.insert(0, "/opt/trn_rl_repo")

DEGREE = 3
NUM_KNOTS = 64
NB = NUM_KNOTS - DEGREE - 1          # 60 basis elements
N_POINTS = 500_000
N_CORES = 8
SHARD = N_POINTS // N_CORES          # 62500
TILE_W = 490                          # points per matmul tile (even: fp32r)
N_TILES = 128                         # 128 * 490 = 62720 >= SHARD
SHARD_PAD = N_TILES * TILE_W          # 62720
NF = 56                               # truncated-power features
FULL_COLS = 3907                      # 128 * 3907 = 500096 >= N_POINTS
FULL_PAD = 128 * FULL_COLS


# ----------------------------------------------------------------- host math
def _piece_poly_coeffs(knots, i, ell):
    """Monomial coeffs (len 4) of the de Boor piece for element i, interval
    ell in [3,6] — replicates the reference recursion, fit exactly in f64."""
    k = DEGREE
    seg = knots[i:i + k + 2]
    T = np.concatenate([np.full(k, seg[0] - 1.0), seg, np.full(k, seg[-1] + 1.0)])

    def eval_at(x):
        res = [np.float64(1.0)] + [np.float64(0.0)] * k
        for j in range(1, k + 1):
            hh = list(res[:j])
            res[0] = np.float64(0.0)
            for n in range(1, j + 1):
                tb, ta = T[ell + n], T[ell + n - j]
                den = tb - ta
                w = 0.0 if den == 0 else hh[n - 1] / den
                res[n - 1] = res[n - 1] + w * (tb - x)
                res[n] = w * (x - ta)
        return res[2 * k - ell]

    xs = np.linspace(-0.3, 1.3, 5)
    V = np.vander(xs, 4, increasing=True)
    return np.linalg.lstsq(V, np.array([eval_at(x) for x in xs]), rcond=None)[0]


def build_tables(knots):
    """A [4,60], CUBE4 [4,56], J [56,60] for the truncated-power form."""
    knots = np.asarray(knots, np.float64)
    P = [[_piece_poly_coeffs(knots, i, p + 3) for p in range(4)] for i in range(NB)]

    def p_of(s, i):
        return int(np.clip(s - i - 1, 0, 3))

    A = np.zeros((4, NB))
    for i in range(NB):
        A[:, i] = P[i][p_of(4, i)]

    ms = list(range(4, 60))
    J = np.zeros((len(ms), NB))
    for f, m in enumerate(ms):
        for i in range(NB):
            pb, pa = p_of(m, i), p_of(m + 1, i)
            if pa != pb:
                J[f, i] = (P[i][pa] - P[i][pb])[3]

    kaps = knots[4:60]
    CUBE4 = np.stack([-kaps**3, 3 * kaps**2, -3 * kaps, np.ones(NF)], 0)
    # reference row at xn == 1.0 exactly: searchsorted gives s = 64 there,
    # so every column evaluates its piece p=3 at 1.0 (a genuine jump for the
    # right-boundary columns); patched on the host for the (rare) argmax hits
    row1 = np.array([np.polyval(P[i][3][::-1], 1.0) for i in range(NB)])
    return A, CUBE4, J, row1


CENTER = 0.5  # powers are of y = xn - CENTER to reduce monomial cancellation


def _shift_poly(c, h):
    """coeffs of p(y + h) given coeffs c of p(x), low->high, exact in f64."""
    from math import comb
    out = np.zeros_like(c)
    for q in range(4):
        for r in range(q + 1):
            out[r] += c[q] * comb(q, r) * h ** (q - r)
    return out


def _make_const_arrays(knots):
    A, CUBE4, J, row1 = build_tables(knots)
    # re-express in y = xn - CENTER
    A = np.stack([_shift_poly(A[:, i], CENTER) for i in range(NB)], 1)
    kaps = np.asarray(knots, np.float64)[4:60] - CENTER
    CUBE4 = np.stack([-kaps**3, 3 * kaps**2, -3 * kaps, np.ones(NF)], 0)
    # Two point-tiles are processed per matmul via block-diagonal weights
    # (fp32r requires output start_partition 0; block-diag gives M=120).
    # MM1 (K=6: powers of tile a, powers of tile b): cols 0-55 produce the
    # non-constant part of (xn-kappa)^3, cols 56-59 pass powers through
    # (constant terms come in via the clamp bias).
    c3 = np.zeros((3, NB), np.float32)
    c3[:, :NF] = CUBE4[1:4, :]
    for q in range(1, 4):
        c3[q - 1, NF + q] = 1.0
    cube3x = np.zeros((6, 2 * NB), np.float32)
    cube3x[0:3, :NB] = c3
    cube3x[3:6, NB:] = c3
    bias = np.zeros((2 * NB, 1), np.float32)
    clampc = np.zeros((2 * NB, 2), np.float32)  # col0: DVE min-clamp, col1: ACT alpha
    clampc[:, 0] = -3.0e38
    for h in (0, NB):
        bias[h:h + NF, 0] = CUBE4[0, :]  # -kappa'^3
        bias[h + NF, 0] = 1.0            # the y^0 == 1 row
        clampc[h:h + NF, 0] = 0.0        # cube rows: relu
        clampc[h + NF:h + NB, 1] = 1.0   # power rows: identity (alpha=1)
    # MM2 weights: rows 0-55 = J band, rows 56-59 = base cubic A, blockdiag
    w2s = np.zeros((NB, NB), np.float32)
    w2s[:NF, :] = J
    w2s[NF:, :] = A
    w2 = np.zeros((2 * NB, 2 * NB), np.float32)
    w2[:NB, :NB] = w2s
    w2[NB:, NB:] = w2s
    return cube3x, bias, clampc, w2, row1


# -------------------------------------------------------------- bass program
_CACHE = {}


def _build_nc():
    import concourse.tile as tile
    from concourse import bacc, mybir

    f32 = mybir.dt.float32
    f32r = mybir.dt.float32r

    nc = bacc.Bacc("TRN2", target_bir_lowering=False, debug=False)
    x_full = nc.declare_dram_parameter("x_full", [128, FULL_COLS], f32, isOutput=False)
    x_shard = nc.declare_dram_parameter("x_shard", [128, TILE_W], f32, isOutput=False)
    cube3x_d = nc.declare_dram_parameter("cube3x", [6, 2 * NB], f32, isOutput=False)
    bias_d = nc.declare_dram_parameter("bias124", [2 * NB, 1], f32, isOutput=False)
    clampc_d = nc.declare_dram_parameter("clampc", [2 * NB, 2], f32, isOutput=False)
    w2_d = nc.declare_dram_parameter("w2", [2 * NB, 2 * NB], f32, isOutput=False)
    out_t = nc.declare_dram_parameter("out_t", [NB, SHARD_PAD], f32, isOutput=True)

    GROUP = 8  # pairs per output staging buffer

    with tile.TileContext(nc) as tc:
        with (
            tc.tile_pool(name="big", bufs=1) as big_pool,
            tc.tile_pool(name="consts", bufs=1) as const_pool,
            tc.tile_pool(name="xrows", bufs=6) as xrows_pool,
            tc.tile_pool(name="stack", bufs=4) as stack_pool,
            tc.tile_pool(name="stage", bufs=3) as stage_pool,
            tc.tile_pool(name="gpsum", bufs=2, space="PSUM") as gpsum_pool,
            tc.tile_pool(name="opsum", bufs=2, space="PSUM") as opsum_pool,
            tc.tile_pool(name="tiny", bufs=1) as tiny_pool,
                    ):
            # ---- constants into SBUF
            cu = const_pool.tile([38, 2 * NB], f32r)
            nc.sync.dma_start(cu[0:6, :], cube3x_d[:, :].bitcast(f32r))
            nc.sync.dma_start(cu[32:38, :], cube3x_d[:, :].bitcast(f32r))
            bias = const_pool.tile([2 * NB, 1], f32)
            nc.sync.dma_start(bias[:, :], bias_d[:, :])
            clampc = const_pool.tile([2 * NB, 2], f32)
            nc.sync.dma_start(clampc[:, :], clampc_d[:, :])
            w2t = const_pool.tile([2 * NB, 2 * NB], f32r)
            nc.sync.dma_start(w2t[:, :], w2_d[:, :].bitcast(f32r))

            # ---- global min/max from the full x (redundant per core)
            # chunked so the reduces overlap the input DMA
            xf = big_pool.tile([128, FULL_COLS], f32)
            NCH = 4
            csz = (FULL_COLS + NCH - 1) // NCH
            pq = tiny_pool.tile([128, 2 * NCH], f32)
            for ci in range(NCH):
                lo = ci * csz
                hi = min(FULL_COLS, lo + csz)
                nc.sync.dma_start(xf[:, lo:hi], x_full[:, lo:hi])
                nc.vector.tensor_reduce(
                    pq[:, ci:ci + 1], xf[:, lo:hi], mybir.AxisListType.X,
                    mybir.AluOpType.min,
                )
                nc.vector.tensor_reduce(
                    pq[:, NCH + ci:NCH + ci + 1], xf[:, lo:hi],
                    mybir.AxisListType.X, mybir.AluOpType.max,
                )
            pm = tiny_pool.tile([128, 2], f32)  # per-partition [-min, max]
            nc.vector.tensor_reduce(
                pm[:, 0:1], pq[:, 0:NCH], mybir.AxisListType.X,
                mybir.AluOpType.min, negate=True,
            )
            nc.vector.tensor_reduce(
                pm[:, 1:2], pq[:, NCH:2 * NCH], mybir.AxisListType.X,
                mybir.AluOpType.max,
            )
            g = tiny_pool.tile([1, 4], f32)  # [min, inv, max, span]
            nc.gpsimd.tensor_reduce(
                g[0:1, 0:1], pm[:, 0:1], mybir.AxisListType.XYZWC,
                mybir.AluOpType.max,
            )
            nc.gpsimd.tensor_reduce(
                g[0:1, 2:3], pm[:, 1:2], mybir.AxisListType.XYZWC,
                mybir.AluOpType.max,
            )
            # g0 currently holds -min: span = (max + (-min)) + 1e-8
            nc.vector.tensor_scalar(
                g[0:1, 3:4], g[0:1, 2:3], g[0:1, 0:1], 1e-8,
                mybir.AluOpType.add, mybir.AluOpType.add,
            )
            nc.vector.reciprocal(g[0:1, 1:2], g[0:1, 3:4])
            # g0 := center = 0.5*span - (-min)  (powers are of y = xn - 0.5)
            nc.vector.scalar_tensor_tensor(
                g[0:1, 0:1], g[0:1, 3:4], 0.5, g[0:1, 0:1],
                mybir.AluOpType.mult, mybir.AluOpType.subtract,
            )
            # broadcast (min, inv) to all partitions via a K=1 matmul
            ones = tiny_pool.tile([1, 128], f32)
            nc.vector.memset(ones[:, :], 1.0)
            muinv_p = gpsum_pool.tile([128, 2], f32, tag="gp")
            nc.tensor.matmul(muinv_p[:, :], ones[:, :], g[0:1, 0:2])
            muinv = tiny_pool.tile([128, 2], f32)
            nc.scalar.copy(muinv[:, :], muinv_p[:, :])

            # ---- power table XP [128, 4*489]: blocks [1 | xn | xn^2 | xn^3]
            xs = big_pool.tile([128, TILE_W], f32)
            nc.sync.dma_start(xs[:, :], x_shard[:, :])
            xp = big_pool.tile([128, 3 * TILE_W], f32)
            W = TILE_W
            nc.vector.tensor_scalar(
                xp[:, 0:W], xs[:, :], muinv[:, 0:1], muinv[:, 1:2],
                mybir.AluOpType.subtract, mybir.AluOpType.mult,
            )
            nc.vector.tensor_mul(xp[:, W:2 * W], xp[:, 0:W], xp[:, 0:W])
            nc.vector.tensor_mul(xp[:, 2 * W:3 * W], xp[:, W:2 * W], xp[:, 0:W])

            # ---- main pipeline: 64 pairs of 490-point tiles (block-diag),
            # two pairs share one 2-bank PSUM tile so clamp/evict batch up
            n_pairs = N_TILES // 2
            BANK = 512
            for g in range(0, n_pairs, GROUP):
                stage = stage_pool.tile([2 * NB, GROUP * W], f32)
                for pb in range(g, min(g + GROUP, n_pairs), 2):
                    sl = (pb - g) * W

                    xr = xrows_pool.tile([38, W], f32r)
                    for h in (0, 1):
                        t0 = 2 * (pb + h)
                        eng = nc.gpsimd if (pb + h) % 2 == 0 else nc.sync
                        eng.dma_start(
                            xr[32 * h:32 * h + 6, :],
                            xp[t0:t0 + 2, :].bitcast(f32r).rearrange(
                                "p (q c) -> p q c", q=3),
                        )

                    gp = gpsum_pool.tile([2 * NB, 2 * BANK], f32)
                    nc.tensor.matmul(gp[:, 0:W], cu[0:6, :], xr[0:6, :])
                    nc.tensor.matmul(
                        gp[:, BANK:BANK + W], cu[32:38, :], xr[32:38, :]
                    )

                    stk = stack_pool.tile([2 * NB, 2 * W], f32r)
                    nc.vector.tensor_scalar(
                        stk[:, :].rearrange("r (p c) -> r p c", c=W),
                        gp[:, :].rearrange("r (p c) -> r p c", c=BANK)[:, :, 0:W],
                        bias[:, :], clampc[:, 0:1],
                        mybir.AluOpType.add, mybir.AluOpType.max,
                    )

                    op = opsum_pool.tile([2 * NB, 2 * BANK], f32)
                    nc.tensor.matmul(op[:, 0:W], w2t[:, :], stk[:, 0:W])
                    nc.tensor.matmul(
                        op[:, BANK:BANK + W], w2t[:, :], stk[:, W:2 * W]
                    )

                    nc.scalar.activation(
                        stage[:, sl:sl + 2 * W].rearrange(
                            "r (p c) -> r p c", c=W),
                        op[:, :].rearrange("r (p c) -> r p c", c=BANK)[:, :, 0:W],
                        mybir.ActivationFunctionType.Copy,
                    )

                # pair-major blocks (even tiles then odd tiles); the host
                # un-interleaves columns, keeping these DMAs fully contiguous
                gw = min(GROUP, n_pairs - g)
                c0 = 2 * g * W
                nc.scalar.dma_start(
                    out_t[:, c0:c0 + gw * W], stage[0:NB, 0:gw * W]
                )
                nc.scalar.dma_start(
                    out_t[:, c0 + gw * W:c0 + 2 * gw * W],
                    stage[NB:2 * NB, 0:gw * W],
                )

    nc.compile()
    return nc


# ------------------------------------------------------------------- driver
def _run(in_maps, trace=False):
    from concourse.bass_utils import run_bass_kernel_spmd

    if "nc" not in _CACHE:
        _CACHE["nc"] = _build_nc()
    return run_bass_kernel_spmd(
        _CACHE["nc"], in_maps, list(range(N_CORES)), trace=trace
    )


def _default_knots():
    inner = np.linspace(0.0, 1.0, NUM_KNOTS - 2 * DEGREE)
    return np.concatenate(
        [np.zeros(DEGREE), inner, np.ones(DEGREE)]
    ).astype(np.float32)


def kernel(x, knots=None, degree=None, _trace=False, _return_results=False, **_):
    x = np.asarray(x, np.float32).reshape(-1)
    assert x.size == N_POINTS
    if knots is None:
        knots = _default_knots()
    cube3x, bias124, clampc, w2, row1 = _make_const_arrays(
        np.asarray(knots, np.float64))

    xf = np.empty(FULL_PAD, np.float32)
    xf[:N_POINTS] = x
    xf[N_POINTS:] = x[0]
    xf = xf.reshape(128, FULL_COLS)

    in_maps = []
    for c in range(N_CORES):
        sh = np.empty(SHARD_PAD, np.float32)
        sh[:SHARD] = x[c * SHARD:(c + 1) * SHARD]
        sh[SHARD:] = x[c * SHARD]
        in_maps.append({
            "x_full": xf,
            "x_shard": sh.reshape(128, TILE_W),
            "cube3x": cube3x,
            "bias124": bias124,
            "clampc": clampc,
            "w2": w2,
        })

    res = _run(in_maps, trace=_trace)
    # device column -> local point index (pair-major group blocks)
    GROUP = 8
    W = TILE_W
    n_pairs = N_TILES // 2
    perm = np.empty(SHARD_PAD, np.int64)
    col = 0
    for g in range(0, n_pairs, GROUP):
        gw = min(GROUP, n_pairs - g)
        for half in (0, 1):
            for pp in range(gw):
                t = 2 * (g + pp) + half
                perm[col:col + W] = t * W + np.arange(W)
                col += W
    out = np.empty((N_POINTS, NB), np.float32)
    full = np.empty((SHARD_PAD, NB), np.float32)
    for c in range(N_CORES):
        full[perm, :] = res.results[c]["out_t"].T
        out[c * SHARD:(c + 1) * SHARD, :] = full[:SHARD]

    # boundary fixup: at xn == 1.0 exactly the reference jumps to the
    # degenerate right-end pieces (s = 64); patch those rows exactly
    mn, mx = x.min(), x.max()
    xn = (x - mn) / ((mx - mn) + np.float32(1e-8))
    at_one = np.nonzero(xn == np.float32(1.0))[0]
    if at_one.size:
        out[at_one, :] = row1.astype(np.float32)[None, :]

    if _return_results:
        return out, res
    return out



# revision 2
# speedup vs baseline: 1.3634x; 1.3634x over previous
"""Trainium2 Bass kernel for BSplineBasis (degree-3, 64 clamped uniform knots).

Math (per normalized point xn, y = xn - 0.5):
    out[n, i] = A_i(y) + sum_m J[m, i] * relu(y - kappa_m)^3
with A_i the base cubic and J a banded jump table (truncated-power form of
the reference's de Boor evaluation, exact for cubics; fit on the host in f64).

Device mapping (the key trick vs the previous version): points are
pre-scaled by a FIXED u = x/8 on device, so the power table, the layout
rearrange and both matmuls are independent of the global min/max; the
data-dependent normalization y = a*u + b (a = 8*inv, b = -min*inv - 0.5)
is folded into the tiny MM1 weight table and bias on device:
    cu'[q,f] = a^q * (C0 + C1*b + C2*b^2)[q,f]      (Horner, [6,120] DVE ops)
    bias'[f] = B0 + B1*b + B2*b^2 + B3*b^3          (Horner, [120,1] ACT ops)
Stack rows per tile: 56 relu'd cube features + 4 power rows shifted to be
relu-safe (1, y+1, y^2, y^3+1), so the clamp is a uniform Relu(G + bias)
and can run on ACT (activation), DVE or Pool (tensor_scalar) -- the two
per-element passes (relu, PSUM->bf16 evict) are load-balanced across all
three engines. Output is written bf16 (error budget 2e-2 >> bf16 noise).

Pipeline per 16-tile chunk: one rearrange DMA xp[16t,1470] -> xr[48,490]
(flat orders match), then 4 iterations x {2 MM1 (K=6), relu, 2 MM2 (K=120),
evict-to-bf16}; output DMAs alternate SP queue / Pool SWDGE to spread the
~1.3us sequencer hold each HWDGE DMA costs.

Each core redundantly computes the global min/max from the full x (the
collective cost model has a 15us constant overhead, so redundancy wins).
Host patches the rare xn == 1.0 rows exactly (reference jumps there).
"""
import os
import sys

import numpy as np

if "/opt/trn_rl_repo" not in sys.path:
    sys.path.insert(0, "/opt/trn_rl_repo")

DEGREE = 3
NUM_KNOTS = 64
NB = NUM_KNOTS - DEGREE - 1          # 60 basis elements
NF = 56                               # truncated-power features
N_POINTS = 500_000
N_CORES = 8
SHARD = N_POINTS // N_CORES          # 62500
TILE_W = 490                          # points per matmul tile
N_TILES = 128                         # 128 * 490 = 62720 >= SHARD
SHARD_PAD = N_TILES * TILE_W          # 62720
FULL_COLS = 3907                      # 128 * 3907 = 500096 >= N_POINTS
FULL_PAD = 128 * FULL_COLS
N_ITER = 32                           # iterations (2 tile-pairs each)
GROUPS = (8, 8, 8, 8)                 # iterations per output-DMA group


# ----------------------------------------------------------------- host math
def _piece_poly_coeffs(knots, i, ell):
    """Monomial coeffs (len 4) of the de Boor piece for element i, interval
    ell in [3,6] -- replicates the reference recursion, fit exactly in f64."""
    k = DEGREE
    seg = knots[i:i + k + 2]
    T = np.concatenate([np.full(k, seg[0] - 1.0), seg, np.full(k, seg[-1] + 1.0)])

    def eval_at(x):
        res = [np.float64(1.0)] + [np.float64(0.0)] * k
        for j in range(1, k + 1):
            hh = list(res[:j])
            res[0] = np.float64(0.0)
            for n in range(1, j + 1):
                tb, ta = T[ell + n], T[ell + n - j]
                den = tb - ta
                w = 0.0 if den == 0 else hh[n - 1] / den
                res[n - 1] = res[n - 1] + w * (tb - x)
                res[n] = w * (x - ta)
        return res[2 * k - ell]

    xs = np.linspace(-0.3, 1.3, 5)
    V = np.vander(xs, 4, increasing=True)
    return np.linalg.lstsq(V, np.array([eval_at(x) for x in xs]), rcond=None)[0]


def build_tables(knots):
    """A [4,60] (y-monomials), J [56,60], row1 [60] from the knots, in f64."""
    knots = np.asarray(knots, np.float64)
    P = [[_piece_poly_coeffs(knots, i, p + 3) for p in range(4)] for i in range(NB)]

    def p_of(s, i):
        return int(np.clip(s - i - 1, 0, 3))

    A = np.zeros((4, NB))
    for i in range(NB):
        A[:, i] = P[i][p_of(4, i)]

    J = np.zeros((NF, NB))
    for f, m in enumerate(range(4, 60)):
        for i in range(NB):
            pb, pa = p_of(m, i), p_of(m + 1, i)
            if pa != pb:
                J[f, i] = (P[i][pa] - P[i][pb])[3]

    row1 = np.array([np.polyval(P[i][3][::-1], 1.0) for i in range(NB)])
    return A, J, row1


def _shift_poly(c, h):
    from math import comb
    out = np.zeros_like(c)
    for q in range(4):
        for r in range(q + 1):
            out[r] += c[q] * comb(q, r) * h ** (q - r)
    return out


def _make_const_arrays(knots):
    """cpack [6, 360] (C0|C1|C2), btab [120, 4] (B0..B3), w2 [120,120]."""
    A, J, row1 = build_tables(knots)
    A = np.stack([_shift_poly(A[:, i], 0.5) for i in range(NB)], 1)  # y-monomials
    kap = np.asarray(knots, np.float64)[4:60] - 0.5                  # y-units

    # stack-row weight tables: cu'[q,f] = a^q*(C0 + C1 b + C2 b^2)[q,f]
    # rows of C*: 0-2 = (u,u^2,u^3) for the even tile (cols 0-59), 3-5 odd.
    C = np.zeros((3, 3, NB))   # [qrow][b-power][col]
    B = np.zeros((4, NB))      # [b-power][col], bias' = sum B[j] b^j
    C[0, 0, :NF] = 3 * kap**2; C[0, 1, :NF] = -6 * kap; C[0, 2, :NF] = 3
    C[1, 0, :NF] = -3 * kap;   C[1, 1, :NF] = 3
    C[2, 0, :NF] = 1
    B[0, :NF] = -kap**3; B[1, :NF] = 3 * kap**2; B[2, :NF] = -3 * kap; B[3, :NF] = 1
    # power rows: p0 = 1, p1 = y+1, p2 = y^2, p3 = y^3+1 (all relu-safe)
    B[0, 56] = 1
    C[0, 0, 57] = 1; B[0, 57] = 1; B[1, 57] = 1
    C[0, 1, 58] = 2; C[1, 0, 58] = 1; B[2, 58] = 1
    C[0, 2, 59] = 3; C[1, 1, 59] = 3; C[2, 0, 59] = 1; B[0, 59] = 1; B[3, 59] = 1

    w2s = np.zeros((NB, NB))
    w2s[:NF, :] = J
    w2s[56, :] = A[0] - A[1] - A[3]   # const row absorbs the p1/p3 +1 shifts
    w2s[57, :] = A[1]
    w2s[58, :] = A[2]
    w2s[59, :] = A[3]

    cpack = np.zeros((6, 360), np.float32)
    # stack-row order is (q, t2): row 2q+t2 = u^(q+1) of the even (t2=0) /
    # odd (t2=1) tile of the pair; even tile feeds stack rows 0-59.
    for j in range(3):
        for q in range(3):
            cpack[2 * q + 0, j * 120:j * 120 + 60] = C[q, j, :]
            cpack[2 * q + 1, j * 120 + 60:j * 120 + 120] = C[q, j, :]
    btab = np.ascontiguousarray(np.tile(B.T, (2, 1)).astype(np.float32))
    w2 = np.zeros((2 * NB, 2 * NB), np.float32)
    w2[:NB, :NB] = w2s
    w2[NB:, NB:] = w2s
    return cpack, btab, w2, row1


# -------------------------------------------------------------- bass program
_CACHE = {}


def _p_sched():
    """Greedy balance of the 64 relu/evict passes across ACT/DVE/Pool.

    Per-pass engine costs (ns, incl. per-instr overheads) and an initial
    Pool debit for its SWDGE output-DMA desc-gen work.
    """
    # GPSIMD cannot access PSUM, so only ACT and DVE can run these passes
    cost = {"A": 544.0, "D": 559.0}
    busy = {"A": 0.0, "D": 0.0}
    out = []
    for _ in range(128):
        e = min(cost, key=lambda k: busy[k] + cost[k])
        busy[e] += cost[e]
        out.append(e)
    return out


def _build_nc():
    import concourse.tile as tile
    from concourse import bacc, mybir

    f32 = mybir.dt.float32
    f32r = mybir.dt.float32r
    bf16 = mybir.dt.bfloat16

    nc = bacc.Bacc("TRN2", target_bir_lowering=False, debug=False)
    x_full = nc.declare_dram_parameter("x_full", [128, FULL_COLS], f32, isOutput=False)
    x_shard = nc.declare_dram_parameter("x_shard", [128, TILE_W], f32, isOutput=False)
    cpack_d = nc.declare_dram_parameter("cpack", [6, 360], f32, isOutput=False)
    btab_d = nc.declare_dram_parameter("btab", [2 * NB, 4], f32, isOutput=False)
    w2_d = nc.declare_dram_parameter("w2", [2 * NB, 2 * NB], f32, isOutput=False)
    out_t = nc.declare_dram_parameter("out_t", [NB, SHARD_PAD], bf16,
                                      isOutput=True)
    HALF = SHARD_PAD // 2  # columns per row-half block of out_t

    sched = _p_sched()
    BANK = 512
    W = TILE_W

    with tile.TileContext(nc) as tc:
        with (
            tc.tile_pool(name="consts", bufs=1) as const_pool,
            tc.tile_pool(name="big", bufs=1) as big_pool,
            tc.tile_pool(name="xr", bufs=2) as xr_pool,
            tc.tile_pool(name="stk", bufs=3) as stk_pool,
            tc.tile_pool(name="stage", bufs=2) as stage_pool,
            tc.tile_pool(name="gpsum", bufs=2, space="PSUM") as gpsum_pool,
            tc.tile_pool(name="opsum", bufs=2, space="PSUM") as opsum_pool,
            # gp_a/gp_b (and op_a/op_b) are separate per-name rings, so
            # bufs=2 gives 4 banks per pool -- 8 banks total

            tc.tile_pool(name="tiny", bufs=1) as tiny_pool,
        ):
            # ---- global min/max from the full x (redundant per core).
            # 6 SWDGE chunk loads; chunks 0-3 free-reduced on DVE, chunks
            # 4-5 reduced straight to scalars on Pool (XYZWC), combined at
            # the end -- balances both engines against the DMA arrival rate.
            xf = big_pool.tile([128, FULL_COLS], f32)
            NCH = 4
            edges = [0, 977, 1955, 2931, 3907]
            pq = tiny_pool.tile([128, 8], f32)
            scal = tiny_pool.tile([1, 4], f32)  # [max c1, max c2, max c3, -]
            for ci in range(NCH):
                lo, hi = edges[ci], edges[ci + 1]
                nc.gpsimd.dma_start(xf[:, lo:hi], x_full[:, lo:hi])
            # input + const DMAs queue behind the x_full chunks on the DMA
            # engines -- x is the critical path, these have slack
            xsp = big_pool.tile([128, W], f32)
            nc.sync.dma_start(xsp[:, :], x_shard[:, :])
            w2t = const_pool.tile([2 * NB, 2 * NB], f32r)
            nc.sync.dma_start(w2t[:, :], w2_d[:, :].bitcast(f32r))
            cp = const_pool.tile([6, 360], f32)
            nc.scalar.dma_start(cp[:, :], cpack_d[:, :])
            bw = const_pool.tile([2 * NB, 4], f32)
            nc.scalar.dma_start(bw[:, :], btab_d[:, :])

            # ---- power table xp [128, 3*490] = [u | u^2 | u^3], u = x/8.
            # u, u^2 on ACT; u^3 on DVE ahead of the reduce backlog (it
            # gates the xr q2 rearrange DMA).
            xp = big_pool.tile([128, 3 * W], f32)
            nc.scalar.activation(
                xp[:, 0:W], xsp[:, :], mybir.ActivationFunctionType.Copy,
                bias=0.0, scale=0.125)
            nc.scalar.square(xp[:, W:2 * W], xp[:, 0:W])
            nc.vector.tensor_mul(xp[:, 2 * W:3 * W], xp[:, W:2 * W], xp[:, 0:W])

            # chunk reduces: DVE does every min (free-axis, negated) plus
            # c0's max; Pool XYZWC-max (cross-lane supports max only)
            # handles c1-c3's max straight to scalars
            for ci in range(NCH):
                lo, hi = edges[ci], edges[ci + 1]
                nc.vector.tensor_reduce(
                    pq[:, ci:ci + 1], xf[:, lo:hi], mybir.AxisListType.X,
                    mybir.AluOpType.min, negate=True,
                )
                if ci == 0:
                    nc.vector.tensor_reduce(
                        pq[:, 4:5], xf[:, lo:hi],
                        mybir.AxisListType.X, mybir.AluOpType.max,
                    )
                else:
                    nc.gpsimd.tensor_reduce(
                        scal[0:1, ci - 1:ci], xf[:, lo:hi],
                        mybir.AxisListType.XYZWC, mybir.AluOpType.max,
                    )

            g = tiny_pool.tile([1, 4], f32)  # [-min, inv, max, span]
            nc.gpsimd.tensor_reduce(
                g[0:1, 0:1], pq[:, 0:4], mybir.AxisListType.XYZWC,
                mybir.AluOpType.max,
            )
            nc.gpsimd.tensor_reduce(
                g[0:1, 2:3], pq[:, 4:5], mybir.AxisListType.XYZWC,
                mybir.AluOpType.max,
            )
            # fold the Pool max scalars in
            nc.vector.tensor_scalar(
                g[0:1, 2:3], g[0:1, 2:3], scal[0:1, 0:1], scal[0:1, 1:2],
                mybir.AluOpType.max, mybir.AluOpType.max,
            )
            nc.vector.tensor_scalar(
                g[0:1, 2:3], g[0:1, 2:3], scal[0:1, 2:3], None,
                mybir.AluOpType.max,
            )
            nc.vector.tensor_scalar(
                g[0:1, 3:4], g[0:1, 2:3], g[0:1, 0:1], 1e-8,
                mybir.AluOpType.add, mybir.AluOpType.add,
            )
            nc.vector.reciprocal(g[0:1, 1:2], g[0:1, 3:4])

            # r8 = [a, a, a^2, a^2, a^3, a^3, b, -] on partition 0
            # (apow order matches the (q, t2) stack-row order)
            r8 = tiny_pool.tile([1, 8], f32)
            nc.vector.tensor_scalar(
                r8[0:1, 0:1], g[0:1, 1:2], 8.0, None, mybir.AluOpType.mult,
            )
            nc.vector.tensor_scalar(
                r8[0:1, 6:7], g[0:1, 0:1], g[0:1, 1:2], 0.5,
                mybir.AluOpType.mult, mybir.AluOpType.subtract,
            )
            nc.vector.tensor_mul(r8[0:1, 2:3], r8[0:1, 0:1], r8[0:1, 0:1])
            nc.vector.tensor_mul(r8[0:1, 4:5], r8[0:1, 2:3], r8[0:1, 0:1])
            ev = r8[0:1, 0:6].rearrange("p (a b) -> p a b", b=2)
            nc.vector.tensor_scalar_add(ev[:, :, 1:2], ev[:, :, 0:1], 0.0)

            # broadcasts: apow6 [6,1] via K=1 transpose-matmul; bb [120,1]
            ones1 = tiny_pool.tile([1, 128], f32)
            nc.vector.memset(ones1[:, :], 1.0)
            bc_ps = gpsum_pool.tile([2 * NB, 8], f32, tag="ga")
            nc.tensor.matmul(bc_ps[0:6, 0:1], r8[0:1, 0:6], ones1[0:1, 0:1])
            nc.tensor.matmul(bc_ps[:, 1:2], ones1[0:1, 0:120], r8[0:1, 6:7])
            apow6 = tiny_pool.tile([6, 1], f32)
            nc.scalar.copy(apow6[:, :], bc_ps[0:6, 0:1])
            bb = tiny_pool.tile([2 * NB, 1], f32)
            nc.scalar.copy(bb[:, :], bc_ps[:, 1:2])

            # cu' Horner on [6, 120] (DVE)
            cuA = const_pool.tile([6, 120], f32r)
            tcu = tiny_pool.tile([6, 120], f32)
            nc.vector.tensor_scalar(
                tcu[:, :], cp[:, 240:360], bb[0:6, 0:1], None,
                mybir.AluOpType.mult,
            )
            nc.vector.tensor_tensor(
                tcu[:, :], tcu[:, :], cp[:, 120:240], mybir.AluOpType.add)
            nc.vector.tensor_scalar(
                tcu[:, :], tcu[:, :], bb[0:6, 0:1], None, mybir.AluOpType.mult)
            nc.vector.tensor_tensor(
                tcu[:, :], tcu[:, :], cp[:, 0:120], mybir.AluOpType.add)
            nc.vector.tensor_scalar(
                cuA[:, :], tcu[:, :], apow6[:, 0:1], None, mybir.AluOpType.mult)

            # bias' Horner on [120, 1] (ACT: out = in*scale + bias)
            bias = const_pool.tile([2 * NB, 1], f32)
            nc.scalar.activation(
                bias[:, :], bw[:, 3:4], mybir.ActivationFunctionType.Identity,
                bias=bw[:, 2:3], scale=bb[:, 0:1])
            nc.scalar.activation(
                bias[:, :], bias[:, :], mybir.ActivationFunctionType.Identity,
                bias=bw[:, 1:2], scale=bb[:, 0:1])
            nc.scalar.activation(
                bias[:, :], bias[:, :], mybir.ActivationFunctionType.Identity,
                bias=bw[:, 0:1], scale=bb[:, 0:1])

            w2r = w2t[:, :]

            # ---- whole-run xr [6, 64*490]: row 2q+t2 = u^(q+1) of the
            # even/odd tile of each pair, pairs along the free dim. 3 DMAs
            # per half (one per power q); flat orders match because the host
            # packs partitions as (half, t2, pair) -- see tile_of_part.
            xr = big_pool.tile([6, 64 * W], f32r)

            def emit_xr_half(h):
                for q in range(3):
                    queue = nc.scalar if q == 1 else nc.sync
                    queue.dma_start(
                        xr[2 * q:2 * q + 2, h * 32 * W:(h + 1) * 32 * W],
                        xp[64 * h:64 * h + 64,
                           q * W:(q + 1) * W].bitcast(f32r),
                    )

            emit_xr_half(0)

            # ---- main pipeline, software-pipelined so the in-order PE queue
            # never head-of-line blocks: iteration i emits MM1(i) then
            # MM2(i-1). P-passes are per-PSUM-bank (halved latency, 4-deep
            # rings on both PSUM pools).
            pslot = 0
            group_of = []
            for gi, gn in enumerate(GROUPS):
                group_of += [gi] * gn
            g_start = np.cumsum([0] + list(GROUPS))
            stages = {}
            prev = None  # (stk, op_a, op_b, stage_views) of iteration i-1

            def emit_p(view_out, view_in, relu):
                nonlocal pslot
                e = sched[pslot]; pslot += 1
                if relu:
                    if e == "A":
                        nc.scalar.activation(
                            view_out, view_in,
                            mybir.ActivationFunctionType.Relu,
                            bias=bias[:, 0:1])
                    else:
                        eng = nc.vector if e == "D" else nc.gpsimd
                        eng.tensor_scalar(
                            view_out, view_in, bias[:, 0:1], 0.0,
                            mybir.AluOpType.add, mybir.AluOpType.max)
                else:
                    if e == "A":
                        nc.scalar.activation(
                            view_out, view_in,
                            mybir.ActivationFunctionType.Copy)
                    else:
                        eng = nc.vector if e == "D" else nc.gpsimd
                        eng.tensor_scalar(
                            view_out, view_in, 0.0, None,
                            mybir.AluOpType.add)

            def emit_mm2_and_evict(state):
                stk_p, st_v, grp = state
                op_a = opsum_pool.tile([2 * NB, BANK], f32, tag="oa")
                nc.tensor.matmul(op_a[:, 0:W], w2r, stk_p[:, 0:W])
                op_b = opsum_pool.tile([2 * NB, BANK], f32, tag="ob")
                nc.tensor.matmul(op_b[:, 0:W], w2r, stk_p[:, W:2 * W])
                emit_p(st_v[:, 0:W], op_a[:, 0:W], relu=False)
                emit_p(st_v[:, W:2 * W], op_b[:, 0:W], relu=False)
                # after the last evict of a group (or half-group for the
                # final group, to shrink the drain tail), fire output DMAs
                if grp is not None:
                    gi, stage, part = grp
                    s0 = 0 if part in (None, 0) else 4 * 2 * W
                    gw = GROUPS[gi] * 2 * W if part is None else 4 * 2 * W
                    c0 = int(g_start[gi]) * 2 * W + s0
                    q1, q2 = ((nc.gpsimd, nc.gpsimd) if gi < 3
                              else (nc.sync, nc.gpsimd))
                    q1.dma_start(out_t[:, c0:c0 + gw],
                                 stage[0:NB, s0:s0 + gw])
                    q2.dma_start(
                        out_t[:, HALF + c0:HALF + c0 + gw],
                        stage[NB:2 * NB, s0:s0 + gw])

            # j = double-iteration (2 adjacent pairs = one 980-col matmul)
            for j in range(N_ITER):
                if j == 16:
                    emit_xr_half(1)
                gi = group_of[j]
                slot = j - int(g_start[gi])
                if slot == 0:
                    stage = stage_pool.tile(
                        [2 * NB, 8 * 2 * W], bf16, name="stage")
                    stages[gi] = stage
                stage = stages[gi]

                gp_a = gpsum_pool.tile([2 * NB, BANK], f32, tag="ga")
                nc.tensor.matmul(gp_a[:, 0:W], cuA[:, :],
                                 xr[0:6, (2 * j) * W:(2 * j + 1) * W])
                gp_b = gpsum_pool.tile([2 * NB, BANK], f32, tag="gb")
                nc.tensor.matmul(gp_b[:, 0:W], cuA[:, :],
                                 xr[0:6, (2 * j + 1) * W:(2 * j + 2) * W])

                stk = stk_pool.tile([2 * NB, 2 * W], f32r)
                emit_p(stk[:, 0:W], gp_a[:, 0:W], relu=True)
                emit_p(stk[:, W:2 * W], gp_b[:, 0:W], relu=True)

                sl = slot * 2 * W
                st_v = stage[:, sl:sl + 2 * W]
                if gi == len(GROUPS) - 1 and slot in (3, GROUPS[gi] - 1):
                    grp = (gi, stage, 0 if slot == 3 else 1)
                elif gi < len(GROUPS) - 1 and slot == GROUPS[gi] - 1:
                    grp = (gi, stage, None)
                else:
                    grp = None
                if prev is not None:
                    emit_mm2_and_evict(prev)
                prev = (stk, st_v, grp)
            emit_mm2_and_evict(prev)

    nc.compile()
    return nc


# ------------------------------------------------------------------- driver
def _run(in_maps, trace=False):
    from concourse.bass_utils import run_bass_kernel_spmd

    if "nc" not in _CACHE:
        _CACHE["nc"] = _build_nc()
    return run_bass_kernel_spmd(
        _CACHE["nc"], in_maps, list(range(N_CORES)), trace=trace
    )


def _default_knots():
    inner = np.linspace(0.0, 1.0, NUM_KNOTS - 2 * DEGREE)
    return np.concatenate(
        [np.zeros(DEGREE), inner, np.ones(DEGREE)]
    ).astype(np.float32)


def _out_perm():
    """device out_t column -> local point index, per row-half.

    out_t is [60, 2*SHARD_PAD]: cols [0, SHARD_PAD) = row-half 0 (even tile
    of each pair), cols [SHARD_PAD, 2*SHARD_PAD) = row-half 1 (odd tiles).
    Iteration i handles tiles 4i..4i+3 as pairs (4i,4i+1), (4i+2,4i+3); its
    columns land at it_off = i*980 + pairidx*490 + c.
    """
    perm = np.empty((2, SHARD_PAD // 2), np.int64)
    c = np.arange(TILE_W)
    for i in range(N_ITER):
        for pairidx in range(2):
            base = i * 2 * TILE_W + pairidx * TILE_W
            for half in range(2):
                tile_id = 4 * i + 2 * pairidx + half
                perm[half, base:base + TILE_W] = tile_id * TILE_W + c
    return perm


def kernel(x, knots=None, degree=None, _trace=False, _return_results=False, **_):
    x = np.asarray(x, np.float32).reshape(-1)
    assert x.size == N_POINTS
    if knots is None:
        knots = _default_knots()
    key = hash(np.asarray(knots, np.float64).tobytes())
    if _CACHE.get("tbl_key") != key:
        _CACHE["tbl"] = _make_const_arrays(np.asarray(knots, np.float64))
        _CACHE["tbl_key"] = key
    cpack, btab, w2, row1 = _CACHE["tbl"]

    xf = np.empty(FULL_PAD, np.float32)
    xf[:N_POINTS] = x
    xf[N_POINTS:] = x[0]
    xf = xf.reshape(128, FULL_COLS)

    # xs partition p holds tile(p): layout (half, t2, pair-in-half) so each
    # xp -> xr rearrange DMA (per power q, per half) is a flat-order-
    # preserving copy: partition = 64*half + 32*t2 + pb, pair = 32*half + pb
    parts = np.arange(128)
    Hh, r = parts // 64, parts % 64
    t2, pb = r // 32, r % 32
    tile_of_part = 2 * (32 * Hh + pb) + t2

    in_maps = []
    for c in range(N_CORES):
        sh = np.empty(SHARD_PAD, np.float32)
        sh[:SHARD] = x[c * SHARD:(c + 1) * SHARD]
        sh[SHARD:] = x[c * SHARD]
        sh = sh.reshape(128, TILE_W)[tile_of_part]  # pack tiles per layout
        in_maps.append({
            "x_full": xf,
            "x_shard": np.ascontiguousarray(sh),
            "cpack": cpack,
            "btab": btab,
            "w2": w2,
        })

    res = _run(in_maps, trace=_trace)

    perm = _out_perm()
    out = np.empty((N_POINTS, NB), np.float32)
    full = np.empty((SHARD_PAD, NB), np.float32)
    half = SHARD_PAD // 2
    for c in range(N_CORES):
        o = np.asarray(res.results[c]["out_t"]).astype(np.float32)  # [60, PAD]
        full[perm[0], :] = o[:, :half].T
        full[perm[1], :] = o[:, half:].T
        out[c * SHARD:(c + 1) * SHARD, :] = full[:SHARD]

    # boundary fixup: at xn == 1.0 exactly the reference jumps to the
    # degenerate right-end pieces; patch those rows exactly
    mn, mx = x.min(), x.max()
    xn = (x - mn) / ((mx - mn) + np.float32(1e-8))
    at_one = np.nonzero(xn == np.float32(1.0))[0]
    if at_one.size:
        out[at_one, :] = row1.astype(np.float32)[None, :]

    if _return_results:
        return out, res
    return out
